# revision 1
# baseline (speedup 1.0000x reference)
"""Multi-head causal self-attention (B=2, S=2048, D=1024, H=16) on 8 TRN2 NeuronCores.

Sharding: data-parallel over batch (2) x tensor-parallel over heads (4 groups of
4 heads). Each core computes Q/K/V projections for its 4 heads, causal
flash-style attention (scores kept transposed [k, q] so no on-chip transposes
are needed), and a partial output projection against its row-slice of W_O.
Host sums the 4 partials per batch and adds the output bias.

Matmuls run in float32r (TF32-like fast path, ~2.5e-4 end-to-end rel err).
Softmax denominators come from an extra all-ones column appended to V (so the
P@V matmul also produces the row sums); the per-query 1/den is broadcast
across partitions via a DRAM-bounce DMA with a partition-step-0 source AP.
Measured: ~299 us (core-0 exec, 8 cores within ~2%), rel err 2.47e-4.
"""

import contextlib
import sys

import numpy as np

sys.path.insert(0, "/opt/trn_rl_repo")

import concourse.bass as bass  # noqa: E402
import concourse.tile as tile  # noqa: E402
from concourse import bacc, mybir  # noqa: E402
from concourse.bass_utils import run_bass_kernel_spmd  # noqa: E402

F32 = mybir.dt.float32
F32R = mybir.dt.float32r
AF = mybir.ActivationFunctionType
ALU = mybir.AluOpType

B, S, D, H = 2, 2048, 1024, 16
DH = D // H          # 64
TPG = 4              # tensor-parallel groups
HPC = H // TPG       # 4 heads per core
CH = HPC * DH        # 256 channels per core
CHA = CH + HPC       # 260: V channels augmented with a ones column per head
NEG = -1.0e9
N_CORES = 8

_PROG = None  # cached compiled Bass program


def _build_program():
    nc = bacc.Bacc("TRN2", target_bir_lowering=False, debug=False,
                   num_devices=N_CORES)

    xT = nc.dram_tensor("xT", [D, S], F32R, kind="ExternalInput").ap()
    wq = nc.dram_tensor("wq", [D, CH], F32R, kind="ExternalInput").ap()
    wk = nc.dram_tensor("wk", [D, CH], F32R, kind="ExternalInput").ap()
    wv = nc.dram_tensor("wv", [D, CHA], F32R, kind="ExternalInput").ap()
    wo = nc.dram_tensor("wo", [CH, D], F32R, kind="ExternalInput").ap()
    bq = nc.dram_tensor("bq", [128, 2], F32, kind="ExternalInput").ap()
    bk = nc.dram_tensor("bk", [128, 2], F32, kind="ExternalInput").ap()
    bv = nc.dram_tensor("bv", [1, CHA], F32R, kind="ExternalInput").ap()
    tri = nc.dram_tensor("tri", [128, 1024], F32, kind="ExternalInput").ap()
    ones = nc.dram_tensor("ones", [1, 128], F32R, kind="ExternalInput").ap()
    onesf = nc.dram_tensor("onesf", [1, 64], F32, kind="ExternalInput").ap()
    out = nc.dram_tensor("out", [S, D], F32, kind="ExternalOutput").ap()

    NQ = S // 512    # 4 q-blocks of 512
    NT = S // 128    # 16 s-tiles / k-blocks

    with tile.TileContext(nc) as tc, contextlib.ExitStack() as ctx:
        const = ctx.enter_context(tc.tile_pool(name="const", bufs=1))
        qt = const.tile([128, 2, S], F32R)     # Q^T/8 (+bq/8): chunk m = heads 2m,2m+1
        kt = const.tile([128, 2, S], F32R)     # K^T (+bk)
        va = const.tile([128, NT, CHA], F32R)  # V augmented: [s, head-major 65-col blocks]
        otn = const.tile([128, 2, S], F32R)    # normalized attention out, transposed
        tri_t = const.tile([128, 1024], F32)
        ones1 = const.tile([1, 128], F32R)
        ones64f = const.tile([1, 64], F32)
        bq_t = const.tile([128, 2], F32)
        bk_t = const.tile([128, 2], F32)
        bv_t = const.tile([1, CHA], F32R)
        wo_t = const.tile([128, 2, D], F32R)
        wor = wo.rearrange("(a p) n -> a p n", p=128)

        # ---- phase 1: projections --------------------------------------
        with tc.tile_pool(name="proj", bufs=1) as proj, \
             tc.tile_pool(name="pps", bufs=4, space="PSUM") as pps:
            xt = proj.tile([128, 8, S], F32R)
            wq_t = proj.tile([128, 8, CH], F32R)
            wk_t = proj.tile([128, 8, CH], F32R)
            wv_t = proj.tile([128, 8, CHA], F32R)
            xTr = xT.rearrange("(a p) s -> a p s", p=128)
            wqr = wq.rearrange("(a p) c -> a p c", p=128)
            wkr = wk.rearrange("(a p) c -> a p c", p=128)
            wvr = wv.rearrange("(a p) c -> a p c", p=128)
            # leading slice: exactly what the first QK matmul needs
            nc.sync.dma_start(xt[:, 0, 0:512], xTr[0][:, 0:512])
            nc.sync.dma_start(wq_t[:, 0, :], wqr[0])
            nc.sync.dma_start(xt[:, 0, 512:S], xTr[0][:, 512:S])
            nc.sync.dma_start(wk_t[:, 0, :], wkr[0])
            nc.sync.dma_start(wv_t[:, 0, :], wvr[0])
            for c in range(1, 8):
                nc.sync.dma_start(xt[:, c, :], xTr[c])
                nc.sync.dma_start(wq_t[:, c, :], wqr[c])
                nc.sync.dma_start(wk_t[:, c, :], wkr[c])
                nc.sync.dma_start(wv_t[:, c, :], wvr[c])
            # non-projection constants after the projection-critical stream
            nc.sync.dma_start(bq_t, bq)
            nc.sync.dma_start(bk_t, bk)
            nc.sync.dma_start(bv_t, bv)
            nc.sync.dma_start(ones1, ones)
            nc.sync.dma_start(ones64f, onesf)
            nc.sync.dma_start(tri_t, tri)
            for c2 in range(2):
                nc.sync.dma_start(wo_t[:, c2, :], wor[c2])

            def qk_proj(w_t, dst, bias_t, scaled, m):
                # stationary = weight chunk, moving = x^T
                for n in range(NQ):
                    ps = pps.tile([128, 512], F32, tag="ps", name="ps")
                    for c in range(8):
                        nc.tensor.matmul(
                            ps, (w_t[:, c, m * 128:(m + 1) * 128]),
                            (xt[:, c, n * 512:(n + 1) * 512]),
                            start=(c == 0), stop=(c == 7))
                    dstv = dst[:, m, n * 512:(n + 1) * 512]
                    # evacuate on ACT (idle during projections); fold the
                    # 1/sqrt(DH) scale into Q
                    nc.scalar.activation(
                        dstv, ps, AF.Identity, bias=bias_t[:, m:m + 1],
                        scale=0.125 if scaled else 1.0)

            def v_proj():
                # V (not transposed): stationary = x^T tile, moving = wv_aug
                for t in range(NT):
                    ps = pps.tile([128, 512], F32, tag="ps", name="ps")
                    psv = ps[:, 0:CHA]
                    for c in range(8):
                        nc.tensor.matmul(
                            psv, (xt[:, c, t * 128:(t + 1) * 128]),
                            (wv_t[:, c, :]), start=(c == 0), stop=False)
                    # bias row (contains the 1.0 for the ones columns)
                    nc.tensor.matmul(psv, (ones1), (bv_t), start=False,
                                     stop=True)
                    nc.vector.tensor_copy(va[:, t, :], psv)

            # preload the ACT exp table set while ACT is otherwise idle
            nc.scalar.activation(ones64f, ones64f, AF.Exp)

            # pair-0 inputs first so attention can overlap chunk-1 projections
            qk_proj(wq_t, qt, bq_t, True, 0)
            qk_proj(wk_t, kt, bk_t, False, 0)
            v_proj()
            qk_proj(wq_t, qt, bq_t, True, 1)
            qk_proj(wk_t, kt, bk_t, False, 1)

        # ---- phase 2+3: attention with interleaved output projection ---
        with tc.tile_pool(name="sm", bufs=4) as sm, \
             tc.tile_pool(name="ost", bufs=4) as ost, \
             tc.tile_pool(name="stp", bufs=3, space="PSUM") as stp, \
             tc.tile_pool(name="pvp", bufs=2, space="PSUM") as pvp, \
             tc.tile_pool(name="dsp", bufs=4, space="DRAM") as dsp:
            for j in range(NQ):         # q-block of 512
                nkb = 4 * (j + 1)       # causal: k-blocks 0..nkb-1
                qsl = slice(j * 512, (j + 1) * 512)
                for p in range(2):      # head pair = channel chunk
                    pv = [pvp.tile([65, 512], F32, tag="pv", name=f"pv{_hh}")
                          for _hh in range(2)]
                    for g in range(nkb // 2):
                        st = [stp.tile([128, 1024], F32, tag="st",
                                       name=f"st{_hh}") for _hh in range(2)]
                        for i in range(2):
                            kb = 2 * g + i
                            for hh in range(2):  # packed rows 0-63/64-127
                                oh = hh * 64
                                nc.tensor.matmul(
                                    st[hh][:, i * 512:(i + 1) * 512],
                                    (kt[oh:oh + 64, p,
                                        kb * 128:(kb + 1) * 128]),
                                    (qt[oh:oh + 64, p, qsl]),
                                    start=True, stop=True)
                        for i in range(2):
                            kb = 2 * g + i
                            rel = kb * 128 - j * 512
                            if rel >= 0:
                                # causal staircase bias over cols [0, rel+128)
                                for hh in range(2):
                                    sl = st[hh][:, i * 512:i * 512 + rel + 128]
                                    nc.vector.tensor_add(
                                        sl, sl, tri_t[:, 512 - rel:640])
                        pt = [None, None]
                        for hh in range(2):
                            pt[hh] = sm.tile([128, 1024], F32R, tag="pt",
                                             name=f"pt{hh}")
                            nc.scalar.activation(pt[hh], st[hh], AF.Exp)
                        for i in range(2):
                            kb = 2 * g + i
                            for hh in range(2):
                                h = 2 * p + hh
                                nc.tensor.matmul(
                                    pv[hh], (va[:, kb, h * 65:h * 65 + 65]),
                                    (pt[hh][:, i * 512:(i + 1) * 512]),
                                    start=(kb == 0), stop=(kb == nkb - 1),
                                    skip_group_check=True)
                    for hh in range(2):
                        oh = hh * 64
                        rec = sm.tile([1, 512], F32, tag="rec")
                        nc.vector.reciprocal(rec, pv[hh][64:65, :])
                        # broadcast 1/den across 64 partitions via DRAM bounce
                        drow = dsp.tile([1, 512], F32, tag="ds", name="ds")
                        nc.sync.dma_start(drow, rec)
                        bcast_src = bass.AP(
                            tensor=drow.tensor, offset=drow.offset,
                            ap=[[0, 64]] + list(drow.ap)[1:])
                        bcs = sm.tile([64, 512], F32, tag="bcs")
                        nc.sync.dma_start(bcs, bcast_src)
                        nc.vector.tensor_mul(otn[oh:oh + 64, p, qsl],
                                             pv[hh][0:64, :], bcs)

        # ---- phase 3: output projection (partial; host reduces) --------
        with tc.tile_pool(name="ost2", bufs=4) as ost2, \
             tc.tile_pool(name="ops", bufs=4, space="PSUM") as ops:
            for t in range(NT):
                for n in range(2):
                    ps = ops.tile([128, 512], F32, tag="ops", name="ops")
                    for c2 in range(2):
                        nc.tensor.matmul(
                            ps, (otn[:, c2, t * 128:(t + 1) * 128]),
                            (wo_t[:, c2, n * 512:(n + 1) * 512]),
                            start=(c2 == 0), stop=(c2 == 1))
                    so = ost2.tile([128, 512], F32, tag="so", name="so")
                    nc.vector.tensor_copy(so, ps)
                    nc.sync.dma_start(out[t * 128:(t + 1) * 128,
                                          n * 512:(n + 1) * 512], so)

    nc.compile()
    return nc


def _tri_np():
    # staircase causal bias: tri[kk, x] = NEG if x < 512+kk else 0
    xs = np.arange(1024)[None, :]
    ks = np.arange(128)[:, None]
    return np.where(xs < 512 + ks, np.float32(NEG),
                    np.float32(0.0)).astype(np.float32)


def build_in_maps(x, Wq, bq, Wk, bk, Wv, bv, Wo):
    tri_np = _tri_np()
    ones_np = np.ones((1, 128), dtype=np.float32)
    xT_b = [np.ascontiguousarray(x[b].T) for b in range(B)]
    in_maps = []
    for c in range(N_CORES):
        b, tp = divmod(c, TPG)
        sl = slice(tp * CH, (tp + 1) * CH)
        wv_aug = np.zeros((D, CHA), dtype=np.float32)
        bv_aug = np.zeros((1, CHA), dtype=np.float32)
        for h in range(HPC):
            hsl = slice(tp * CH + h * DH, tp * CH + (h + 1) * DH)
            wv_aug[:, h * 65:h * 65 + DH] = Wv[:, hsl]
            bv_aug[0, h * 65:h * 65 + DH] = bv[hsl]
            bv_aug[0, h * 65 + DH] = 1.0
        in_maps.append({
            "xT": xT_b[b],
            "wq": np.ascontiguousarray(Wq[:, sl], dtype=np.float32),
            "wk": np.ascontiguousarray(Wk[:, sl], dtype=np.float32),
            "wv": wv_aug,
            "wo": np.ascontiguousarray(Wo[sl, :], dtype=np.float32),
            "bq": (bq[sl].astype(np.float32) * 0.125).reshape(2, 128).T.copy(),
            "bk": bk[sl].astype(np.float32).reshape(2, 128).T.copy(),
            "bv": bv_aug,
            "tri": tri_np,
            "ones": ones_np,
            "onesf": ones_np[:, :64].copy(),
        })
    return in_maps


def _get_program():
    global _PROG
    if _PROG is None:
        _PROG = _build_program()
    return _PROG


def kernel(x, mask, Wq, bq, Wk, bk, Wv, bv, Wo, bo):
    x = np.asarray(x, dtype=np.float32)
    mask = np.asarray(mask)
    Wq, Wk, Wv, Wo = (np.asarray(w, dtype=np.float32)
                      for w in (Wq, Wk, Wv, Wo))
    bq, bk, bv, bo = (np.asarray(b, dtype=np.float32)
                      for b in (bq, bk, bv, bo))
    causal = bool(
        np.array_equal(mask != 0,
                       np.tril(np.ones((S, S), dtype=bool))))
    if not causal:
        # Fallback for non-causal masks: exact host computation.
        q = (x @ Wq + bq).reshape(B, S, H, DH).transpose(0, 2, 1, 3)
        k = (x @ Wk + bk).reshape(B, S, H, DH).transpose(0, 2, 1, 3)
        v = (x @ Wv + bv).reshape(B, S, H, DH).transpose(0, 2, 1, 3)
        attn = np.einsum("bhqd,bhkd->bhqk", q, k) / np.sqrt(np.float32(DH))
        attn = np.where(mask == 0, np.float32(-1e9), attn)
        attn = attn - attn.max(axis=-1, keepdims=True)
        e = np.exp(attn)
        p = e / e.sum(axis=-1, keepdims=True)
        o = np.einsum("bhqk,bhkd->bhqd", p, v)
        o = o.transpose(0, 2, 1, 3).reshape(B, S, D)
        return (o @ Wo + bo).astype(np.float32)

    nc = _get_program()
    in_maps = build_in_maps(x, Wq, bq, Wk, bk, Wv, bv, Wo)
    res = run_bass_kernel_spmd(nc, in_maps, core_ids=list(range(N_CORES)))
    out = np.zeros((B, S, D), dtype=np.float32)
    for c in range(N_CORES):
        out[c // TPG] += res.results[c]["out"]
    out += bo.astype(np.float32)
    return out



# revision 3
# speedup vs baseline: 1.0511x; 1.0511x over previous
"""Multi-head causal self-attention (B=2, S=2048, D=1024, H=16) on 8 TRN2 NeuronCores.

Sharding: data-parallel over batch (2) x tensor-parallel over heads (4 groups of
4 heads). Each core computes Q/K/V projections for its 4 heads, causal
flash-style attention (scores kept transposed [k, q] so no on-chip transposes
are needed), and a partial output projection against its row-slice of W_O.
Host sums the 4 partials per batch and adds the output bias.

v2: bf16 matmul operands (less PE power -> less duty-cycle throttle, half the
DMA/LDWEIGHTS bytes), per-k-block software-pipelined attention steps (the PE
queue is [QK(s), PV(s-1)] so PV never waits on the softmax exp), PSUM
rebalanced (4x one-bank score tiles + 2x double-head pv accumulators), and the
softmax normalize chain (psum evac -> den bounce -> [128,8] reciprocal ->
stride-0 broadcast -> multiply) deferred several steps off the critical path.
"""

import contextlib
import sys

import numpy as np

sys.path.insert(0, "/opt/trn_rl_repo")

import concourse.bass as bass  # noqa: E402
import concourse.tile as tile  # noqa: E402
from concourse import bacc, mybir  # noqa: E402
from concourse.bass_utils import run_bass_kernel_spmd  # noqa: E402

F32 = mybir.dt.float32
BF16 = mybir.dt.bfloat16
AF = mybir.ActivationFunctionType
ALU = mybir.AluOpType

B, S, D, H = 2, 2048, 1024, 16
DH = D // H          # 64
TPG = 4              # tensor-parallel groups
HPC = H // TPG       # 4 heads per core
CH = HPC * DH        # 256 channels per core
CHA = CH + HPC       # 260: V channels augmented with a ones column per head
NEG = -1.0e9
N_CORES = 8

_PROG = None  # cached compiled Bass program


def _build_program():
    nc = bacc.Bacc("TRN2", target_bir_lowering=False, debug=False,
                   num_devices=N_CORES)

    xT = nc.dram_tensor("xT", [D, S], BF16, kind="ExternalInput").ap()
    wq = nc.dram_tensor("wq", [D, CH], BF16, kind="ExternalInput").ap()
    wk = nc.dram_tensor("wk", [D, CH], BF16, kind="ExternalInput").ap()
    wv = nc.dram_tensor("wv", [D, CHA], BF16, kind="ExternalInput").ap()
    wo = nc.dram_tensor("wo", [CH, D], BF16, kind="ExternalInput").ap()
    bq = nc.dram_tensor("bq", [128, 2], F32, kind="ExternalInput").ap()
    bk = nc.dram_tensor("bk", [128, 2], F32, kind="ExternalInput").ap()
    bv = nc.dram_tensor("bv", [1, CHA], BF16, kind="ExternalInput").ap()
    tri = nc.dram_tensor("tri", [128, 1024], F32, kind="ExternalInput").ap()
    ones = nc.dram_tensor("ones", [1, 128], BF16, kind="ExternalInput").ap()
    onesf = nc.dram_tensor("onesf", [1, 64], F32, kind="ExternalInput").ap()
    out = nc.dram_tensor("out", [S, D], F32, kind="ExternalOutput").ap()

    NQ = S // 512    # 4 q-blocks of 512
    NT = S // 128    # 16 s-tiles / k-blocks

    with tile.TileContext(nc) as tc, contextlib.ExitStack() as ctx:
        const = ctx.enter_context(tc.tile_pool(name="const", bufs=1))
        qt = const.tile([128, 2, S], BF16)     # Q^T/8 (+bq/8): chunk m = heads 2m,2m+1
        kt = const.tile([128, 2, S], BF16)     # K^T (+bk)
        va = const.tile([128, NT, CHA], BF16)  # V augmented: [s, head-major 65-col blocks]
        otn = const.tile([128, 2, S], BF16)    # normalized attention out, transposed
        tri_t = const.tile([128, 1024], F32)
        ones1 = const.tile([1, 128], BF16)
        ones64f = const.tile([1, 64], F32)
        bq_t = const.tile([128, 2], F32)
        bk_t = const.tile([128, 2], F32)
        bv_t = const.tile([1, CHA], BF16)
        wo_t = const.tile([128, 2, D], BF16)
        wor = wo.rearrange("(a p) n -> a p n", p=128)

        # ---- phase 1: projections --------------------------------------
        with tc.tile_pool(name="proj", bufs=1) as proj, \
             tc.tile_pool(name="pqk", bufs=3, space="PSUM") as pqk, \
             tc.tile_pool(name="pvv", bufs=2, space="PSUM") as pvv:
            xt = proj.tile([128, 8, S], BF16)
            wq_t = proj.tile([128, 8, CH], BF16)
            wk_t = proj.tile([128, 8, CH], BF16)
            wv_t = proj.tile([128, 8, CHA], BF16)
            xTr = xT.rearrange("(a p) s -> a p s", p=128)
            wqr = wq.rearrange("(a p) c -> a p c", p=128)
            wkr = wk.rearrange("(a p) c -> a p c", p=128)
            wvr = wv.rearrange("(a p) c -> a p c", p=128)
            # leading slice: exactly what the first QK matmul needs
            nc.sync.dma_start(xt[:, 0, 0:512], xTr[0][:, 0:512])
            nc.sync.dma_start(wq_t[:, 0, :], wqr[0])
            nc.sync.dma_start(xt[:, 0, 512:S], xTr[0][:, 512:S])
            nc.sync.dma_start(wk_t[:, 0, :], wkr[0])
            nc.sync.dma_start(wv_t[:, 0, :], wvr[0])
            for c in range(1, 8):
                nc.sync.dma_start(xt[:, c, :], xTr[c])
                nc.sync.dma_start(wq_t[:, c, :], wqr[c])
                nc.sync.dma_start(wk_t[:, c, :], wkr[c])
                nc.sync.dma_start(wv_t[:, c, :], wvr[c])
            # non-projection constants after the projection-critical stream
            nc.sync.dma_start(bq_t, bq)
            nc.sync.dma_start(bk_t, bk)
            nc.sync.dma_start(bv_t, bv)
            nc.sync.dma_start(ones1, ones)
            nc.sync.dma_start(ones64f, onesf)
            nc.sync.dma_start(tri_t, tri)
            for c2 in range(2):
                nc.sync.dma_start(wo_t[:, c2, :], wor[c2])

            def qk_proj(w_t, dst, bias_t, scaled, m):
                # stationary = weight chunk, moving = x^T
                for n in range(NQ):
                    ps = pqk.tile([128, 512], F32, tag="pqk", name="pqk")
                    for c in range(8):
                        nc.tensor.matmul(
                            ps, (w_t[:, c, m * 128:(m + 1) * 128]),
                            (xt[:, c, n * 512:(n + 1) * 512]),
                            start=(c == 0), stop=(c == 7))
                    dstv = dst[:, m, n * 512:(n + 1) * 512]
                    # evacuate on ACT (idle during projections); fold the
                    # 1/sqrt(DH) scale into Q
                    nc.scalar.activation(
                        dstv, ps, AF.Identity, bias=bias_t[:, m:m + 1],
                        scale=0.125 if scaled else 1.0)

            def v_proj():
                # V (not transposed): stationary = x^T tile, moving = wv_aug
                for t in range(NT):
                    psv = pvv.tile([128, CHA], F32, tag="pvv", name="pvv")
                    for c in range(8):
                        nc.tensor.matmul(
                            psv, (xt[:, c, t * 128:(t + 1) * 128]),
                            (wv_t[:, c, :]), start=(c == 0), stop=False)
                    # bias row (contains the 1.0 for the ones columns)
                    nc.tensor.matmul(psv, (ones1), (bv_t), start=False,
                                     stop=True)
                    nc.vector.tensor_copy(va[:, t, :], psv)

            # preload the ACT exp table set while ACT is otherwise idle
            nc.scalar.activation(ones64f, ones64f, AF.Exp)

            # pair-0 inputs first so attention can overlap chunk-1 projections
            qk_proj(wq_t, qt, bq_t, True, 0)
            qk_proj(wk_t, kt, bk_t, False, 0)
            v_proj()
            qk_proj(wq_t, qt, bq_t, True, 1)
            qk_proj(wk_t, kt, bk_t, False, 1)

        # ---- phase 2: attention, software-pipelined per k-block --------
        # step = (j, p, kb): one 128-row k-block of scores for a 512-col
        # q-block, both packed head-halves (hh).  The PE queue is
        # [QK(s), PV(s-1)] so PV never waits on exp latency.  The softmax
        # normalize chain for each (j, p) unit is deferred several steps.
        steps = []
        for j in range(NQ):
            for p in range(2):
                nkb = 4 * (j + 1)
                for kb in range(nkb):
                    steps.append((j, p, kb, kb == 0, kb == nkb - 1))
        n_steps = len(steps)

        deferred = []   # (due_step, fn), non-decreasing due order

        def run_due(s):
            while deferred and deferred[0][0] <= s:
                deferred.pop(0)[1]()

        with tc.tile_pool(name="ovp", bufs=2) as ovp, \
             tc.tile_pool(name="rcp", bufs=2) as rcp, \
             tc.tile_pool(name="bcp", bufs=2) as bcp, \
             tc.tile_pool(name="dsp", bufs=4, space="DRAM") as dsp:

            def make_normalize(j, p, pv, s_end):
                # chain: evac pv->SBUF (frees PSUM) -> bounce den ->
                # [128,8] reciprocal -> bounce back -> stride-0 broadcast
                # -> per-head multiply into otn.
                qsl = slice(j * 512, (j + 1) * 512)
                state = {}

                def evac():
                    ov = ovp.tile([65, 1024], F32, tag="ov", name="ov")
                    nc.vector.tensor_copy(ov, pv)
                    drow = dsp.tile([1, 1024], F32, tag="ds", name="ds")
                    nc.sync.dma_start(drow, ov[64:65, :])
                    rin = rcp.tile([128, 2, 4], F32, tag="ri", name="ri")
                    din_src = bass.AP(
                        tensor=drow.tensor, offset=drow.offset,
                        ap=[[4, 128], [512, 2], [1, 4]])
                    nc.sync.dma_start(rin, din_src)
                    state["ov"] = ov
                    state["rin"] = rin

                def recip():
                    rout = rcp.tile([128, 2, 4], F32, tag="ro", name="ro")
                    nc.vector.reciprocal(rout, state["rin"])
                    rrow = dsp.tile([1, 1024], F32, tag="rr", name="rr")
                    rr_dst = bass.AP(
                        tensor=rrow.tensor, offset=rrow.offset,
                        ap=[[4, 128], [512, 2], [1, 4]])
                    nc.sync.dma_start(rr_dst, rout)
                    bcs = bcp.tile([64, 2, 512], F32, tag="bc", name="bc")
                    bc_src = bass.AP(
                        tensor=rrow.tensor, offset=rrow.offset,
                        ap=[[0, 64], [512, 2], [1, 512]])
                    nc.sync.dma_start(bcs, bc_src)
                    state["bcs"] = bcs

                def muls():
                    ov, bcs = state["ov"], state["bcs"]
                    for hh in range(2):
                        oh = hh * 64
                        nc.vector.tensor_mul(
                            otn[oh:oh + 64, p, qsl],
                            ov[0:64, hh * 512:(hh + 1) * 512],
                            bcs[:, hh, :])

                deferred.append((s_end + 2, evac))
                deferred.append((s_end + 3, recip))
                deferred.append((s_end + 5, muls))

            with tc.tile_pool(name="sm", bufs=4) as sm, \
                 tc.tile_pool(name="stp", bufs=4, space="PSUM") as stp, \
                 tc.tile_pool(name="pvp", bufs=2, space="PSUM") as pvp:
                prev = None     # (pt tiles, j, p, kb, first, last, pv)
                pv = None
                for s, (j, p, kb, first, last) in enumerate(steps):
                    run_due(s)
                    if first:
                        pv = pvp.tile([65, 1024], F32, tag="pv", name="pv")
                    qsl = slice(j * 512, (j + 1) * 512)
                    st = [stp.tile([128, 512], F32, tag="st",
                                   name=f"st{_hh}") for _hh in range(2)]
                    for hh in range(2):
                        oh = hh * 64
                        nc.tensor.matmul(
                            st[hh],
                            (kt[oh:oh + 64, p, kb * 128:(kb + 1) * 128]),
                            (qt[oh:oh + 64, p, qsl]),
                            start=True, stop=True)
                    # interleave the previous step's PV behind this QK
                    if prev is not None:
                        _emit_pv(nc, va, prev)
                    rel = kb * 128 - j * 512
                    pt = [None, None]
                    for hh in range(2):
                        if rel >= 0:
                            # causal staircase bias over cols [0, rel+128)
                            sl = st[hh][:, 0:rel + 128]
                            nc.vector.tensor_add(sl, sl,
                                                 tri_t[:, 512 - rel:640])
                        pt[hh] = sm.tile([128, 512], BF16, tag="pt",
                                         name=f"pt{hh}")
                        nc.scalar.activation(pt[hh], st[hh], AF.Exp)
                    prev = (pt, j, p, kb, first, last, pv)
                    if last:
                        make_normalize(j, p, pv, s)
                _emit_pv(nc, va, prev)
                # flush everything due so far (incl. the last unit's psum
                # evac) while the score/pv pools are still open
                run_due(n_steps + 1)

            # ---- phase 3: output projection (partial; host reduces) ----
            with tc.tile_pool(name="ost2", bufs=4) as ost2, \
                 tc.tile_pool(name="ops", bufs=4, space="PSUM") as ops:
                for t in range(NT):
                    if t == 4:
                        run_due(n_steps + 3)
                    if t == 8:
                        run_due(n_steps + 5)
                    for n in range(2):
                        ps = ops.tile([128, 512], F32, tag="ops", name="ops")
                        for c2 in range(2):
                            nc.tensor.matmul(
                                ps, (otn[:, c2, t * 128:(t + 1) * 128]),
                                (wo_t[:, c2, n * 512:(n + 1) * 512]),
                                start=(c2 == 0), stop=(c2 == 1))
                        so = ost2.tile([128, 512], F32, tag="so", name="so")
                        nc.vector.tensor_copy(so, ps)
                        nc.sync.dma_start(out[t * 128:(t + 1) * 128,
                                              n * 512:(n + 1) * 512], so)

    nc.compile()
    return nc


def _emit_pv(nc, va, prev):
    pt, j, p, kb, first, last, pv = prev
    for hh in range(2):
        h = 2 * p + hh
        nc.tensor.matmul(
            pv[:, hh * 512:(hh + 1) * 512],
            (va[:, kb, h * 65:h * 65 + 65]),
            (pt[hh]),
            start=first, stop=last, skip_group_check=True)


def _tri_np():
    # staircase causal bias: tri[kk, x] = NEG if x < 512+kk else 0
    xs = np.arange(1024)[None, :]
    ks = np.arange(128)[:, None]
    return np.where(xs < 512 + ks, np.float32(NEG),
                    np.float32(0.0)).astype(np.float32)


def build_in_maps(x, Wq, bq, Wk, bk, Wv, bv, Wo):
    import ml_dtypes
    bf16 = ml_dtypes.bfloat16
    tri_np = _tri_np()
    ones_np = np.ones((1, 128), dtype=bf16)
    xT_b = [np.ascontiguousarray(x[b].T).astype(bf16) for b in range(B)]
    in_maps = []
    for c in range(N_CORES):
        b, tp = divmod(c, TPG)
        sl = slice(tp * CH, (tp + 1) * CH)
        wv_aug = np.zeros((D, CHA), dtype=np.float32)
        bv_aug = np.zeros((1, CHA), dtype=np.float32)
        for h in range(HPC):
            hsl = slice(tp * CH + h * DH, tp * CH + (h + 1) * DH)
            wv_aug[:, h * 65:h * 65 + DH] = Wv[:, hsl]
            bv_aug[0, h * 65:h * 65 + DH] = bv[hsl]
            bv_aug[0, h * 65 + DH] = 1.0
        in_maps.append({
            "xT": xT_b[b],
            "wq": np.ascontiguousarray(Wq[:, sl]).astype(bf16),
            "wk": np.ascontiguousarray(Wk[:, sl]).astype(bf16),
            "wv": wv_aug.astype(bf16),
            "wo": np.ascontiguousarray(Wo[sl, :]).astype(bf16),
            "bq": (bq[sl].astype(np.float32) * 0.125).reshape(2, 128).T.copy(),
            "bk": bk[sl].astype(np.float32).reshape(2, 128).T.copy(),
            "bv": bv_aug.astype(bf16),
            "tri": tri_np,
            "ones": ones_np,
            "onesf": np.ones((1, 64), dtype=np.float32),
        })
    return in_maps


def _get_program():
    global _PROG
    if _PROG is None:
        _PROG = _build_program()
    return _PROG


def kernel(x, mask, Wq, bq, Wk, bk, Wv, bv, Wo, bo):
    x = np.asarray(x, dtype=np.float32)
    mask = np.asarray(mask)
    Wq, Wk, Wv, Wo = (np.asarray(w, dtype=np.float32)
                      for w in (Wq, Wk, Wv, Wo))
    bq, bk, bv, bo = (np.asarray(b, dtype=np.float32)
                      for b in (bq, bk, bv, bo))
    causal = bool(
        np.array_equal(mask != 0,
                       np.tril(np.ones((S, S), dtype=bool))))
    if not causal:
        # Fallback for non-causal masks: exact host computation.
        q = (x @ Wq + bq).reshape(B, S, H, DH).transpose(0, 2, 1, 3)
        k = (x @ Wk + bk).reshape(B, S, H, DH).transpose(0, 2, 1, 3)
        v = (x @ Wv + bv).reshape(B, S, H, DH).transpose(0, 2, 1, 3)
        attn = np.einsum("bhqd,bhkd->bhqk", q, k) / np.sqrt(np.float32(DH))
        attn = np.where(mask == 0, np.float32(-1e9), attn)
        attn = attn - attn.max(axis=-1, keepdims=True)
        e = np.exp(attn)
        p = e / e.sum(axis=-1, keepdims=True)
        o = np.einsum("bhqk,bhkd->bhqd", p, v)
        o = o.transpose(0, 2, 1, 3).reshape(B, S, D)
        return (o @ Wo + bo).astype(np.float32)

    nc = _get_program()
    in_maps = build_in_maps(x, Wq, bq, Wk, bk, Wv, bv, Wo)
    res = run_bass_kernel_spmd(nc, in_maps, core_ids=list(range(N_CORES)))
    out = np.zeros((B, S, D), dtype=np.float32)
    for c in range(N_CORES):
        out[c // TPG] += res.results[c]["out"]
    out += bo.astype(np.float32)
    return out


# revision 11
# speedup vs baseline: 1.1537x; 1.0976x over previous
"""Multi-head causal self-attention (B=2, S=2048, D=1024, H=16) on 8 TRN2 NeuronCores.

Sharding: data-parallel over batch (2) x tensor-parallel over heads (4 groups of
4 heads). Each core computes Q/K/V projections for its 4 heads, causal
flash-style attention (scores kept transposed [k, q] so no on-chip transposes
are needed), and a partial output projection against its row-slice of W_O.
Host sums the 4 partials per batch and adds the output bias.

v2: bf16 matmul operands (less PE power -> less duty-cycle throttle, half the
DMA/LDWEIGHTS bytes), per-k-block software-pipelined attention steps (the PE
queue is [QK(s), PV(s-1)] so PV never waits on the softmax exp), PSUM
rebalanced (4x one-bank score tiles + 2x double-head pv accumulators), and the
softmax normalize chain (psum evac -> den bounce -> [128,8] reciprocal ->
stride-0 broadcast -> multiply) deferred several steps off the critical path.
"""

import contextlib
import sys

import numpy as np

sys.path.insert(0, "/opt/trn_rl_repo")

import concourse.bass as bass  # noqa: E402
import concourse.tile as tile  # noqa: E402
from concourse import bacc, mybir  # noqa: E402
from concourse.bass_utils import run_bass_kernel_spmd  # noqa: E402

F32 = mybir.dt.float32
BF16 = mybir.dt.bfloat16
AF = mybir.ActivationFunctionType
ALU = mybir.AluOpType

B, S, D, H = 2, 2048, 1024, 16
DH = D // H          # 64
TPG = 4              # tensor-parallel groups
HPC = H // TPG       # 4 heads per core
CH = HPC * DH        # 256 channels per core
CHA = CH + HPC       # 260: V channels augmented with a ones column per head
NEG = -1.0e9
N_CORES = 8

_PROG = None  # cached compiled Bass program


def _build_program():
    nc = bacc.Bacc("TRN2", target_bir_lowering=False, debug=False,
                   num_devices=N_CORES)

    xT = nc.dram_tensor("xT", [D, S], BF16, kind="ExternalInput").ap()
    wq = nc.dram_tensor("wq", [D, CH], BF16, kind="ExternalInput").ap()
    wk = nc.dram_tensor("wk", [D, CH], BF16, kind="ExternalInput").ap()
    wv = nc.dram_tensor("wv", [D, CHA], BF16, kind="ExternalInput").ap()
    wo = nc.dram_tensor("wo", [CH, D], BF16, kind="ExternalInput").ap()
    bq = nc.dram_tensor("bq", [128, 2], F32, kind="ExternalInput").ap()
    bk = nc.dram_tensor("bk", [128, 2], F32, kind="ExternalInput").ap()
    tri = nc.dram_tensor("tri", [128, 1024], F32, kind="ExternalInput").ap()
    onesf = nc.dram_tensor("onesf", [1, 64], F32, kind="ExternalInput").ap()
    out = nc.dram_tensor("out", [S, D], F32, kind="ExternalOutput").ap()

    NQ = S // 512    # 4 q-blocks of 512
    NT = S // 128    # 16 s-tiles / k-blocks

    with tile.TileContext(nc) as tc, contextlib.ExitStack() as ctx:
        const = ctx.enter_context(tc.tile_pool(name="const", bufs=1))
        qt = const.tile([128, 2, S], BF16)     # Q^T/8 (+bq/8): chunk m = heads 2m,2m+1
        kt = const.tile([128, 2, S], BF16)     # K^T (+bk)
        va = const.tile([128, NT, CHA], BF16)  # V augmented: [s, head-major 65-col blocks]
        otn = const.tile([128, 2, S], BF16)    # normalized attention out, transposed
        tri_t = const.tile([128, 1024], F32)
        ones64f = const.tile([1, 64], F32)
        bq_t = const.tile([128, 2], F32)
        bk_t = const.tile([128, 2], F32)
        wo_t = const.tile([128, 2, D], BF16)
        wor = wo.rearrange("(a p) n -> a p n", p=128)

        # ---- phase 1: projections --------------------------------------
        with tc.tile_pool(name="proj", bufs=1) as proj, \
             tc.tile_pool(name="pqk", bufs=3, space="PSUM") as pqk, \
             tc.tile_pool(name="pvv", bufs=2, space="PSUM") as pvv:
            xt = proj.tile([128, 8, S], BF16)
            wq_t = proj.tile([128, 8, CH], BF16)
            wk_t = proj.tile([128, 8, CH], BF16)
            wv_t = proj.tile([128, 8, CHA], BF16)
            xTr = xT.rearrange("(a p) s -> a p s", p=128)
            wqr = wq.rearrange("(a p) c -> a p c", p=128)
            wkr = wk.rearrange("(a p) c -> a p c", p=128)
            wvr = wv.rearrange("(a p) c -> a p c", p=128)
            # column-block-major x stream: the first projection matmul group
            # needs only x[:, 0:512] (all 8 row-chunks) + wq, ~1.5 MB
            for c in range(8):
                nc.sync.dma_start(xt[:, c, 0:512], xTr[c][:, 0:512])
            for c in range(8):
                nc.sync.dma_start(wq_t[:, c, :], wqr[c])
            nc.sync.dma_start(bq_t, bq)
            for c in range(8):
                nc.sync.dma_start(wk_t[:, c, :], wkr[c])
            nc.sync.dma_start(bk_t, bk)
            for n in range(1, NQ):
                for c in range(8):
                    nc.sync.dma_start(xt[:, c, n * 512:(n + 1) * 512],
                                      xTr[c][:, n * 512:(n + 1) * 512])
            for c in range(8):
                nc.sync.dma_start(wv_t[:, c, :], wvr[c])
            # non-projection constants after the projection-critical stream
            nc.sync.dma_start(ones64f, onesf)
            nc.sync.dma_start(tri_t, tri)
            for c2 in range(2):
                nc.sync.dma_start(wo_t[:, c2, :], wor[c2])

            def qk_proj(w_t, dst, bias_t, scaled, m):
                # stationary = weight chunk, moving = x^T
                for n in range(NQ):
                    ps = pqk.tile([128, 512], F32, tag="pqk", name="pqk")
                    for c in range(8):
                        nc.tensor.matmul(
                            ps, (w_t[:, c, m * 128:(m + 1) * 128]),
                            (xt[:, c, n * 512:(n + 1) * 512]),
                            start=(c == 0), stop=(c == 7))
                    dstv = dst[:, m, n * 512:(n + 1) * 512]
                    # evacuate on ACT (idle during projections); fold the
                    # 1/sqrt(DH) scale into Q
                    nc.scalar.activation(
                        dstv, ps, AF.Identity, bias=bias_t[:, m:m + 1],
                        scale=0.125 if scaled else 1.0)

            def v_proj():
                # V (not transposed): stationary = x^T tile, moving = wv_aug
                # (the V bias folds into the host-side output bias because
                # softmax weights sum to 1: o_norm += bv  =>  out += bv @ Wo)
                for t in range(NT):
                    psv = pvv.tile([128, CHA], F32, tag="pvv", name="pvv")
                    for c in range(8):
                        nc.tensor.matmul(
                            psv, (xt[:, c, t * 128:(t + 1) * 128]),
                            (wv_t[:, c, :]), start=(c == 0), stop=(c == 7))
                    nc.vector.tensor_copy(va[:, t, :], psv)
                    # the per-head denominator "ones" columns (65th of each
                    # head block; zero in wv_aug so psv has zeros there)
                    ones_view = bass.AP(
                        tensor=va.tensor, offset=va.offset + t * CHA + DH,
                        ap=[list(va.ap)[0], [DH + 1, HPC]])
                    nc.vector.memset(ones_view, 1.0)

            # preload the ACT exp table set while ACT is otherwise idle
            nc.scalar.activation(ones64f, ones64f, AF.Exp)

            # pair-0 inputs first so attention can overlap chunk-1 projections
            qk_proj(wq_t, qt, bq_t, True, 0)
            qk_proj(wk_t, kt, bk_t, False, 0)
            v_proj()
            qk_proj(wq_t, qt, bq_t, True, 1)
            qk_proj(wk_t, kt, bk_t, False, 1)

        # ---- phase 2: attention, software-pipelined per k-block --------
        # step = (j, p, kb): one 128-row k-block of scores for a 512-col
        # q-block, both packed head-halves (hh).  The PE queue is
        # [QK(s), PV(s-1)] so PV never waits on exp latency.  The softmax
        # normalize chain for each (j, p) unit is deferred several steps.
        steps = []
        for j in range(NQ):
            for p in range(2):
                nkb = 4 * (j + 1)
                for kb in range(nkb):
                    steps.append((j, p, kb, kb == 0, kb == nkb - 1))
        n_steps = len(steps)

        deferred = []   # (due_step, fn), non-decreasing due order

        def run_due(s):
            while deferred and deferred[0][0] <= s:
                deferred.pop(0)[1]()

        with tc.tile_pool(name="ovp", bufs=2) as ovp, \
             tc.tile_pool(name="rcp", bufs=2) as rcp, \
             tc.tile_pool(name="bcp", bufs=2) as bcp, \
             tc.tile_pool(name="dsp", bufs=4, space="DRAM") as dsp:

            def make_normalize(j, p, pv, s_end):
                # chain: evac pv->SBUF (frees PSUM) -> bounce den ->
                # [128,8] reciprocal -> bounce back -> stride-0 broadcast
                # -> per-head multiply into otn.
                qsl = slice(j * 512, (j + 1) * 512)
                state = {}

                def evac():
                    ov = ovp.tile([65, 1024], F32, tag="ov", name="ov")
                    nc.vector.tensor_copy(ov, pv)
                    drow = dsp.tile([1, 1024], F32, tag="ds", name="ds")
                    nc.sync.dma_start(drow, ov[64:65, :])
                    rin = rcp.tile([128, 2, 4], F32, tag="ri", name="ri")
                    din_src = bass.AP(
                        tensor=drow.tensor, offset=drow.offset,
                        ap=[[4, 128], [512, 2], [1, 4]])
                    nc.sync.dma_start(rin, din_src)
                    state["ov"] = ov
                    state["rin"] = rin

                def recip():
                    rout = rcp.tile([128, 2, 4], F32, tag="ro", name="ro")
                    nc.vector.reciprocal(rout, state["rin"])
                    rrow = dsp.tile([1, 1024], F32, tag="rr", name="rr")
                    rr_dst = bass.AP(
                        tensor=rrow.tensor, offset=rrow.offset,
                        ap=[[4, 128], [512, 2], [1, 4]])
                    nc.sync.dma_start(rr_dst, rout)
                    bcs = bcp.tile([64, 2, 512], F32, tag="bc", name="bc")
                    bc_src = bass.AP(
                        tensor=rrow.tensor, offset=rrow.offset,
                        ap=[[0, 64], [512, 2], [1, 512]])
                    nc.sync.dma_start(bcs, bc_src)
                    state["bcs"] = bcs

                def muls():
                    ov, bcs = state["ov"], state["bcs"]
                    for hh in range(2):
                        oh = hh * 64
                        nc.vector.tensor_mul(
                            otn[oh:oh + 64, p, qsl],
                            ov[0:64, hh * 512:(hh + 1) * 512],
                            bcs[:, hh, :])

                deferred.append((s_end + 2, evac))
                deferred.append((s_end + 3, recip))
                deferred.append((s_end + 5, muls))

            with tc.tile_pool(name="sm", bufs=4) as sm, \
                 tc.tile_pool(name="stp", bufs=4, space="PSUM") as stp, \
                 tc.tile_pool(name="pvp", bufs=2, space="PSUM") as pvp:
                prev = None     # (pt tiles, j, p, kb, first, last, pv, rel)
                pv = None
                for s, (j, p, kb, first, last) in enumerate(steps):
                    run_due(s)
                    if first:
                        pv = pvp.tile([65, 1024], F32, tag="pv", name="pv")
                    # causal trim: the diagonal k-block kb only attends to
                    # q in [rel, 512) of this q-block; compute only that.
                    rel = max(kb * 128 - j * 512, 0)
                    w = 512 - rel
                    qsl = slice(j * 512 + rel, (j + 1) * 512)
                    st = [stp.tile([128, 512], F32, tag="st",
                                   name=f"st{_hh}") for _hh in range(2)]
                    for hh in range(2):
                        oh = hh * 64
                        nc.tensor.matmul(
                            st[hh][:, 0:w],
                            (kt[oh:oh + 64, p, kb * 128:(kb + 1) * 128]),
                            (qt[oh:oh + 64, p, qsl]),
                            start=True, stop=True)
                    # interleave the previous step's PV behind this QK
                    if prev is not None:
                        _emit_pv(nc, va, prev)
                    diag = kb * 128 - j * 512 >= 0
                    pt = [None, None]
                    for hh in range(2):
                        if diag:
                            # causal staircase bias over the first 128 cols
                            sl = st[hh][:, 0:128]
                            nc.vector.tensor_add(sl, sl, tri_t[:, 512:640])
                        pt[hh] = sm.tile([128, 512], BF16, tag="pt",
                                         name=f"pt{hh}")
                        nc.scalar.activation(pt[hh][:, 0:w], st[hh][:, 0:w],
                                             AF.Exp)
                    prev = (pt, j, p, kb, first, last, pv, rel, w)
                    if last:
                        make_normalize(j, p, pv, s)
                _emit_pv(nc, va, prev)
                # flush everything due so far (incl. the last unit's psum
                # evac) while the score/pv pools are still open
                run_due(n_steps + 1)

            # ---- phase 3: output projection (partial; host reduces) ----
            with tc.tile_pool(name="ost2", bufs=4) as ost2, \
                 tc.tile_pool(name="ops", bufs=4, space="PSUM") as ops:
                for t in range(NT):
                    if t == 4:
                        run_due(n_steps + 3)
                    if t == 8:
                        run_due(n_steps + 5)
                    for n in range(2):
                        ps = ops.tile([128, 512], F32, tag="ops", name="ops")
                        for c2 in range(2):
                            nc.tensor.matmul(
                                ps, (otn[:, c2, t * 128:(t + 1) * 128]),
                                (wo_t[:, c2, n * 512:(n + 1) * 512]),
                                start=(c2 == 0), stop=(c2 == 1))
                        so = ost2.tile([128, 512], F32, tag="so", name="so")
                        nc.vector.tensor_copy(so, ps)
                        nc.sync.dma_start(out[t * 128:(t + 1) * 128,
                                              n * 512:(n + 1) * 512], so)

    nc.compile()
    return nc


def _emit_pv(nc, va, prev):
    pt, j, p, kb, first, last, pv, rel, w = prev
    for hh in range(2):
        h = 2 * p + hh
        nc.tensor.matmul(
            pv[:, hh * 512 + rel:(hh + 1) * 512],
            (va[:, kb, h * 65:h * 65 + 65]),
            (pt[hh][:, 0:w]),
            start=first, stop=last, skip_group_check=True)


def _tri_np():
    # staircase causal bias: tri[kk, x] = NEG if x < 512+kk else 0
    xs = np.arange(1024)[None, :]
    ks = np.arange(128)[:, None]
    return np.where(xs < 512 + ks, np.float32(NEG),
                    np.float32(0.0)).astype(np.float32)


def build_in_maps(x, Wq, bq, Wk, bk, Wv, bv, Wo):
    import ml_dtypes
    bf16 = ml_dtypes.bfloat16
    tri_np = _tri_np()
    xT_b = [np.ascontiguousarray(x[b].T).astype(bf16) for b in range(B)]
    in_maps = []
    for c in range(N_CORES):
        b, tp = divmod(c, TPG)
        sl = slice(tp * CH, (tp + 1) * CH)
        # V bias is folded into the host-side output bias (bv @ Wo); the
        # ones columns are memset on-device.
        wv_aug = np.zeros((D, CHA), dtype=np.float32)
        for h in range(HPC):
            hsl = slice(tp * CH + h * DH, tp * CH + (h + 1) * DH)
            wv_aug[:, h * 65:h * 65 + DH] = Wv[:, hsl]
        in_maps.append({
            "xT": xT_b[b],
            "wq": np.ascontiguousarray(Wq[:, sl]).astype(bf16),
            "wk": np.ascontiguousarray(Wk[:, sl]).astype(bf16),
            "wv": wv_aug.astype(bf16),
            "wo": np.ascontiguousarray(Wo[sl, :]).astype(bf16),
            "bq": (bq[sl].astype(np.float32) * 0.125).reshape(2, 128).T.copy(),
            "bk": bk[sl].astype(np.float32).reshape(2, 128).T.copy(),
            "tri": tri_np,
            "onesf": np.ones((1, 64), dtype=np.float32),
        })
    return in_maps


def _get_program():
    global _PROG
    if _PROG is None:
        _PROG = _build_program()
    return _PROG


def kernel(x, mask, Wq, bq, Wk, bk, Wv, bv, Wo, bo):
    x = np.asarray(x, dtype=np.float32)
    mask = np.asarray(mask)
    Wq, Wk, Wv, Wo = (np.asarray(w, dtype=np.float32)
                      for w in (Wq, Wk, Wv, Wo))
    bq, bk, bv, bo = (np.asarray(b, dtype=np.float32)
                      for b in (bq, bk, bv, bo))
    causal = bool(
        np.array_equal(mask != 0,
                       np.tril(np.ones((S, S), dtype=bool))))
    if not causal:
        # Fallback for non-causal masks: exact host computation.
        q = (x @ Wq + bq).reshape(B, S, H, DH).transpose(0, 2, 1, 3)
        k = (x @ Wk + bk).reshape(B, S, H, DH).transpose(0, 2, 1, 3)
        v = (x @ Wv + bv).reshape(B, S, H, DH).transpose(0, 2, 1, 3)
        attn = np.einsum("bhqd,bhkd->bhqk", q, k) / np.sqrt(np.float32(DH))
        attn = np.where(mask == 0, np.float32(-1e9), attn)
        attn = attn - attn.max(axis=-1, keepdims=True)
        e = np.exp(attn)
        p = e / e.sum(axis=-1, keepdims=True)
        o = np.einsum("bhqk,bhkd->bhqd", p, v)
        o = o.transpose(0, 2, 1, 3).reshape(B, S, D)
        return (o @ Wo + bo).astype(np.float32)

    nc = _get_program()
    in_maps = build_in_maps(x, Wq, bq, Wk, bk, Wv, bv, Wo)
    res = run_bass_kernel_spmd(nc, in_maps, core_ids=list(range(N_CORES)))
    out = np.zeros((B, S, D), dtype=np.float32)
    for c in range(N_CORES):
        out[c // TPG] += res.results[c]["out"]
    # softmax weights sum to 1, so the V bias contributes exactly bv @ Wo
    out += (bv.astype(np.float32) @ Wo) + bo.astype(np.float32)
    return out


# revision 17
# speedup vs baseline: 1.2429x; 1.0773x over previous
"""Multi-head causal self-attention (B=2, S=2048, D=1024, H=16) on 8 TRN2 NeuronCores.

Sharding: data-parallel over batch (2) x tensor-parallel over heads (4 groups of
4 heads). Each core computes Q/K/V projections for its 4 heads, causal
flash-style attention (scores kept transposed [k, q] so no on-chip transposes
are needed), and a partial output projection against its row-slice of W_O.
Host sums the 4 partials per batch and adds the output bias.

v2: bf16 matmul operands (less PE power -> less duty-cycle throttle, half the
DMA/LDWEIGHTS bytes), per-k-block software-pipelined attention steps (the PE
queue is [QK(s), PV(s-1)] so PV never waits on the softmax exp), PSUM
rebalanced (4x one-bank score tiles + 2x double-head pv accumulators), and the
softmax normalize chain (psum evac -> den bounce -> [128,8] reciprocal ->
stride-0 broadcast -> multiply) deferred several steps off the critical path.
"""

import contextlib
import sys

import numpy as np

sys.path.insert(0, "/opt/trn_rl_repo")

import concourse.bass as bass  # noqa: E402
import concourse.tile as tile  # noqa: E402
from concourse import bacc, mybir  # noqa: E402
from concourse.bass_utils import run_bass_kernel_spmd  # noqa: E402

F32 = mybir.dt.float32
BF16 = mybir.dt.bfloat16
AF = mybir.ActivationFunctionType
ALU = mybir.AluOpType

B, S, D, H = 2, 2048, 1024, 16
DH = D // H          # 64
TPG = 4              # tensor-parallel groups
HPC = H // TPG       # 4 heads per core
CH = HPC * DH        # 256 channels per core
CHA = CH + HPC       # 260: V channels augmented with a ones column per head
NEG = -1.0e9
N_CORES = 8

_PROG = None  # cached compiled Bass program


def _build_program():
    nc = bacc.Bacc("TRN2", target_bir_lowering=False, debug=False,
                   num_devices=N_CORES)

    # weights arrive chunk-interleaved ([partition, chunk, col]) so each
    # DMA packet is a full 4KB per-partition row
    xT = nc.dram_tensor("xT", [D, S], BF16, kind="ExternalInput").ap()
    wq = nc.dram_tensor("wq", [128, 8 * CH], BF16, kind="ExternalInput").ap()
    wk = nc.dram_tensor("wk", [128, 8 * CH], BF16, kind="ExternalInput").ap()
    wv = nc.dram_tensor("wv", [128, 8 * CHA], BF16, kind="ExternalInput").ap()
    wo = nc.dram_tensor("wo", [128, 2 * D], BF16, kind="ExternalInput").ap()
    bq = nc.dram_tensor("bq", [128, 2], F32, kind="ExternalInput").ap()
    bk = nc.dram_tensor("bk", [128, 2], F32, kind="ExternalInput").ap()
    tri = nc.dram_tensor("tri", [128, 1024], F32, kind="ExternalInput").ap()
    onesf = nc.dram_tensor("onesf", [1, 64], F32, kind="ExternalInput").ap()
    out = nc.dram_tensor("out", [S, D], F32, kind="ExternalOutput").ap()

    NQ = S // 512    # 4 q-blocks of 512
    NT = S // 128    # 16 s-tiles / k-blocks

    with tile.TileContext(nc) as tc, contextlib.ExitStack() as ctx:
        const = ctx.enter_context(tc.tile_pool(name="const", bufs=1))
        qt = const.tile([128, 2, S], BF16)     # Q^T/8 (+bq/8): chunk m = heads 2m,2m+1
        kt = const.tile([128, 2, S], BF16)     # K^T (+bk)
        va = const.tile([128, NT, CHA], BF16)  # V augmented: [s, head-major 65-col blocks]
        otn = const.tile([128, 2, S], BF16)    # normalized attention out, transposed
        tri_t = const.tile([128, 1024], F32)
        ones64f = const.tile([1, 64], F32)
        bq_t = const.tile([128, 2], F32)
        bk_t = const.tile([128, 2], F32)
        wo_t = const.tile([128, 2, D], BF16)

        # ---- phase 1: projections --------------------------------------
        with tc.tile_pool(name="proj", bufs=1) as proj, \
             tc.tile_pool(name="pqk", bufs=8, space="PSUM") as pqk:
            xt = proj.tile([128, 8, S], BF16)
            wq_t = proj.tile([128, 8, CH], BF16)
            wk_t = proj.tile([128, 8, CH], BF16)
            wv_t = proj.tile([128, 8, CHA], BF16)
            xTr = xT.rearrange("(a p) s -> a p s", p=128)
            # x streams chunk-by-chunk (4KB per-partition packets); the
            # c-outer matmul loop below starts after x chunk 0 + wq only.
            nc.sync.dma_start(xt[:, 0, :], xTr[0])
            nc.sync.dma_start(wq_t, wq)
            nc.sync.dma_start(bq_t, bq)
            nc.sync.dma_start(xt[:, 1, :], xTr[1])
            nc.sync.dma_start(wk_t, wk)
            nc.sync.dma_start(bk_t, bk)
            for c in range(2, 8):
                nc.sync.dma_start(xt[:, c, :], xTr[c])
            nc.sync.dma_start(wv_t, wv)
            # non-projection constants after the projection-critical stream
            nc.sync.dma_start(ones64f, onesf)
            nc.sync.dma_start(tri_t, tri)
            nc.sync.dma_start(wo_t, wo)

            # preload the ACT exp table set while ACT is otherwise idle
            nc.scalar.activation(ones64f, ones64f, AF.Exp)

            def qk_pass(m):
                # c-outer: 8 open psum groups (Q n0..3, K n0..3) accumulate
                # as each x chunk arrives; first matmul needs only chunk 0
                groups = []
                for w_t, dst, bias_t, scaled in ((wq_t, qt, bq_t, True),
                                                 (wk_t, kt, bk_t, False)):
                    for n in range(NQ):
                        ps = pqk.tile([128, 512], F32, tag="pqk", name="pqk")
                        groups.append((ps, w_t, dst, bias_t, scaled, n))
                for c in range(8):
                    for ps, w_t, dst, bias_t, scaled, n in groups:
                        nc.tensor.matmul(
                            ps, (w_t[:, c, m * 128:(m + 1) * 128]),
                            (xt[:, c, n * 512:(n + 1) * 512]),
                            start=(c == 0), stop=(c == 7))
                        if c == 7:
                            # evacuate on ACT; fold 1/sqrt(DH) into Q
                            nc.scalar.activation(
                                dst[:, m, n * 512:(n + 1) * 512], ps,
                                AF.Identity, bias=bias_t[:, m:m + 1],
                                scale=0.125 if scaled else 1.0)

            def v_proj():
                # V (not transposed): stationary = x^T tile, moving = wv_aug
                # (the V bias folds into the host-side output bias because
                # softmax weights sum to 1: o_norm += bv  =>  out += bv @ Wo)
                for t in range(NT):
                    psv = pqk.tile([128, CHA], F32, tag="pqk", name="psv")
                    for c in range(8):
                        nc.tensor.matmul(
                            psv, (xt[:, c, t * 128:(t + 1) * 128]),
                            (wv_t[:, c, :]), start=(c == 0), stop=(c == 7))
                    nc.vector.tensor_copy(va[:, t, :], psv)
                    # the per-head denominator "ones" columns (65th of each
                    # head block; zero in wv_aug so psv has zeros there)
                    ones_view = bass.AP(
                        tensor=va.tensor, offset=va.offset + t * CHA + DH,
                        ap=[list(va.ap)[0], [DH + 1, HPC]])
                    nc.vector.memset(ones_view, 1.0)

            qk_pass(0)
            v_proj()
            qk_pass(1)

        # ---- phase 2: attention, software-pipelined per k-block --------
        # step = (j, p, kb): one 128-row k-block of scores for a 512-col
        # q-block, both packed head-halves (hh).  The PE queue is
        # [QK(s), PV(s-1)] so PV never waits on exp latency.  The softmax
        # normalize chain for each (j, p) unit is deferred several steps.
        steps = []
        for j in range(NQ):
            for p in range(2):
                nkb = 4 * (j + 1)
                for kb in range(nkb):
                    steps.append((j, p, kb, kb == 0, kb == nkb - 1))
        n_steps = len(steps)

        deferred = []   # (due_step, fn), non-decreasing due order

        def run_due(s):
            while deferred and deferred[0][0] <= s:
                deferred.pop(0)[1]()

        with tc.tile_pool(name="ovp", bufs=2) as ovp, \
             tc.tile_pool(name="rcp", bufs=2) as rcp, \
             tc.tile_pool(name="bcp", bufs=2) as bcp, \
             tc.tile_pool(name="dsp", bufs=4, space="DRAM") as dsp:

            def make_normalize(j, p, pv, s_end):
                # chain: evac pv->SBUF (frees PSUM) -> bounce den ->
                # [128,8] reciprocal -> bounce back -> stride-0 broadcast
                # -> per-head multiply into otn.
                qsl = slice(j * 512, (j + 1) * 512)
                state = {}

                def evac():
                    ov = ovp.tile([65, 1024], F32, tag="ov", name="ov")
                    nc.vector.tensor_copy(ov, pv)
                    drow = dsp.tile([1, 1024], F32, tag="ds", name="ds")
                    nc.sync.dma_start(drow, ov[64:65, :])
                    rin = rcp.tile([128, 2, 4], F32, tag="ri", name="ri")
                    din_src = bass.AP(
                        tensor=drow.tensor, offset=drow.offset,
                        ap=[[4, 128], [512, 2], [1, 4]])
                    nc.sync.dma_start(rin, din_src)
                    state["ov"] = ov
                    state["rin"] = rin

                def recip():
                    rout = rcp.tile([128, 2, 4], F32, tag="ro", name="ro")
                    nc.vector.reciprocal(rout, state["rin"])
                    rrow = dsp.tile([1, 1024], F32, tag="rr", name="rr")
                    rr_dst = bass.AP(
                        tensor=rrow.tensor, offset=rrow.offset,
                        ap=[[4, 128], [512, 2], [1, 4]])
                    nc.sync.dma_start(rr_dst, rout)
                    bcs = bcp.tile([64, 2, 512], F32, tag="bc", name="bc")
                    bc_src = bass.AP(
                        tensor=rrow.tensor, offset=rrow.offset,
                        ap=[[0, 64], [512, 2], [1, 512]])
                    nc.sync.dma_start(bcs, bc_src)
                    state["bcs"] = bcs

                def muls():
                    ov, bcs = state["ov"], state["bcs"]
                    for hh in range(2):
                        oh = hh * 64
                        nc.vector.tensor_mul(
                            otn[oh:oh + 64, p, qsl],
                            ov[0:64, hh * 512:(hh + 1) * 512],
                            bcs[:, hh, :])

                deferred.append((s_end + 2, evac))
                deferred.append((s_end + 3, recip))
                deferred.append((s_end + 5, muls))

            with tc.tile_pool(name="sm", bufs=4) as sm, \
                 tc.tile_pool(name="stp", bufs=4, space="PSUM") as stp, \
                 tc.tile_pool(name="pvp", bufs=2, space="PSUM") as pvp:
                prev = None     # (pt tiles, j, p, kb, first, last, pv, rel)
                pv = None
                for s, (j, p, kb, first, last) in enumerate(steps):
                    run_due(s)
                    if first:
                        pv = pvp.tile([65, 1024], F32, tag="pv", name="pv")
                    # causal trim: the diagonal k-block kb only attends to
                    # q in [rel, 512) of this q-block; compute only that.
                    rel = max(kb * 128 - j * 512, 0)
                    w = 512 - rel
                    qsl = slice(j * 512 + rel, (j + 1) * 512)
                    st = [stp.tile([128, 512], F32, tag="st",
                                   name=f"st{_hh}") for _hh in range(2)]
                    for hh in range(2):
                        oh = hh * 64
                        nc.tensor.matmul(
                            st[hh][:, 0:w],
                            (kt[oh:oh + 64, p, kb * 128:(kb + 1) * 128]),
                            (qt[oh:oh + 64, p, qsl]),
                            start=True, stop=True)
                    # interleave the previous step's PV behind this QK
                    if prev is not None:
                        _emit_pv(nc, va, prev)
                    diag = kb * 128 - j * 512 >= 0
                    pt = [None, None]
                    for hh in range(2):
                        if diag:
                            # causal staircase bias over the first 128 cols
                            sl = st[hh][:, 0:128]
                            nc.vector.tensor_add(sl, sl, tri_t[:, 512:640])
                        pt[hh] = sm.tile([128, 512], BF16, tag="pt",
                                         name=f"pt{hh}")
                        nc.scalar.activation(pt[hh][:, 0:w], st[hh][:, 0:w],
                                             AF.Exp)
                    prev = (pt, j, p, kb, first, last, pv, rel, w)
                    if last:
                        make_normalize(j, p, pv, s)
                _emit_pv(nc, va, prev)
                # flush everything due so far (incl. the last unit's psum
                # evac) while the score/pv pools are still open
                run_due(n_steps + 1)

            # ---- phase 3: output projection (partial; host reduces) ----
            with tc.tile_pool(name="ost2", bufs=4) as ost2, \
                 tc.tile_pool(name="ops", bufs=4, space="PSUM") as ops:
                for t in range(NT):
                    if t == 4:
                        run_due(n_steps + 3)
                    if t == 8:
                        run_due(n_steps + 5)
                    # full 4KB output rows per DMA packet: evac both
                    # 512-wide psum halves into one [128, 1024] tile
                    so = ost2.tile([128, 1024], F32, tag="so", name="so")
                    for n in range(2):
                        ps = ops.tile([128, 512], F32, tag="ops", name="ops")
                        for c2 in range(2):
                            nc.tensor.matmul(
                                ps, (otn[:, c2, t * 128:(t + 1) * 128]),
                                (wo_t[:, c2, n * 512:(n + 1) * 512]),
                                start=(c2 == 0), stop=(c2 == 1))
                        nc.vector.tensor_copy(so[:, n * 512:(n + 1) * 512],
                                              ps)
                    nc.sync.dma_start(out[t * 128:(t + 1) * 128, :], so)

    nc.compile()
    return nc


def _emit_pv(nc, va, prev):
    pt, j, p, kb, first, last, pv, rel, w = prev
    for hh in range(2):
        h = 2 * p + hh
        nc.tensor.matmul(
            pv[:, hh * 512 + rel:(hh + 1) * 512],
            (va[:, kb, h * 65:h * 65 + 65]),
            (pt[hh][:, 0:w]),
            start=first, stop=last, skip_group_check=True)


def _tri_np():
    # staircase causal bias: tri[kk, x] = NEG if x < 512+kk else 0
    xs = np.arange(1024)[None, :]
    ks = np.arange(128)[:, None]
    return np.where(xs < 512 + ks, np.float32(NEG),
                    np.float32(0.0)).astype(np.float32)


def build_in_maps(x, Wq, bq, Wk, bk, Wv, bv, Wo):
    import ml_dtypes
    bf16 = ml_dtypes.bfloat16
    tri_np = _tri_np()
    xT_b = [np.ascontiguousarray(x[b].T).astype(bf16) for b in range(B)]
    in_maps = []
    for c in range(N_CORES):
        b, tp = divmod(c, TPG)
        sl = slice(tp * CH, (tp + 1) * CH)
        # V bias is folded into the host-side output bias (bv @ Wo); the
        # ones columns are memset on-device.
        wv_aug = np.zeros((D, CHA), dtype=np.float32)
        for h in range(HPC):
            hsl = slice(tp * CH + h * DH, tp * CH + (h + 1) * DH)
            wv_aug[:, h * 65:h * 65 + DH] = Wv[:, hsl]

        def chunked(w, nch):
            # [nch*128, cols] -> [128, nch*cols]: per-partition rows hold
            # all chunks contiguously so DMA packets are full 4KB rows
            cols = w.shape[1]
            return np.ascontiguousarray(
                w.reshape(nch, 128, cols).transpose(1, 0, 2)
            ).reshape(128, nch * cols).astype(bf16)

        in_maps.append({
            "xT": xT_b[b],
            "wq": chunked(np.ascontiguousarray(Wq[:, sl]), 8),
            "wk": chunked(np.ascontiguousarray(Wk[:, sl]), 8),
            "wv": chunked(wv_aug, 8),
            "wo": chunked(np.ascontiguousarray(Wo[sl, :]), 2),
            "bq": (bq[sl].astype(np.float32) * 0.125).reshape(2, 128).T.copy(),
            "bk": bk[sl].astype(np.float32).reshape(2, 128).T.copy(),
            "tri": tri_np,
            "onesf": np.ones((1, 64), dtype=np.float32),
        })
    return in_maps


def _get_program():
    global _PROG
    if _PROG is None:
        _PROG = _build_program()
    return _PROG


def kernel(x, mask, Wq, bq, Wk, bk, Wv, bv, Wo, bo):
    x = np.asarray(x, dtype=np.float32)
    mask = np.asarray(mask)
    Wq, Wk, Wv, Wo = (np.asarray(w, dtype=np.float32)
                      for w in (Wq, Wk, Wv, Wo))
    bq, bk, bv, bo = (np.asarray(b, dtype=np.float32)
                      for b in (bq, bk, bv, bo))
    causal = bool(
        np.array_equal(mask != 0,
                       np.tril(np.ones((S, S), dtype=bool))))
    if not causal:
        # Fallback for non-causal masks: exact host computation.
        q = (x @ Wq + bq).reshape(B, S, H, DH).transpose(0, 2, 1, 3)
        k = (x @ Wk + bk).reshape(B, S, H, DH).transpose(0, 2, 1, 3)
        v = (x @ Wv + bv).reshape(B, S, H, DH).transpose(0, 2, 1, 3)
        attn = np.einsum("bhqd,bhkd->bhqk", q, k) / np.sqrt(np.float32(DH))
        attn = np.where(mask == 0, np.float32(-1e9), attn)
        attn = attn - attn.max(axis=-1, keepdims=True)
        e = np.exp(attn)
        p = e / e.sum(axis=-1, keepdims=True)
        o = np.einsum("bhqk,bhkd->bhqd", p, v)
        o = o.transpose(0, 2, 1, 3).reshape(B, S, D)
        return (o @ Wo + bo).astype(np.float32)

    nc = _get_program()
    in_maps = build_in_maps(x, Wq, bq, Wk, bk, Wv, bv, Wo)
    res = run_bass_kernel_spmd(nc, in_maps, core_ids=list(range(N_CORES)))
    out = np.zeros((B, S, D), dtype=np.float32)
    for c in range(N_CORES):
        out[c // TPG] += res.results[c]["out"]
    # softmax weights sum to 1, so the V bias contributes exactly bv @ Wo
    out += (bv.astype(np.float32) @ Wo) + bo.astype(np.float32)
    return out
